# revision 2
# baseline (speedup 1.0000x reference)
"""Attention-pooling kernel for TRN2 (8 NeuronCores, data-parallel over batch).

Problem (nn_AttentionPooling3): x [16, 4096, 1024] f32; per head h of 8,
logit[b,h,t] = x[b,t,h*128:(h+1)*128] @ (Q[h] @ key_p[h]) / sqrt(64);
attn = softmax over t; out[b, h*128:(h+1)*128] = sum_t attn * x-slice.

Strategy per core (2 batches/core), measured-rate design:
- The 33.6MB fp32 x stream at the ~410 GB/s per-core HWDGE rate sets an
  ~82us roofline; a ~7.6us framework preamble precedes the first issue.
- x loads are split across BOTH HWDGE queues (qSP ~2/3, qAct ~1/3) and
  issued two units ahead of their convert: the second ring fills the DMA
  pipe during the ramp and the Scalar engine has slack (convert 3.7us +
  exp 0.3us per 4-chunk unit < the 5.1us DMA unit period).
- ScalarE converts each unit to fp16 (1 elem/cycle, ~3.7us/unit) so the
  DVE runs 2x mode and the PE gets a 2-byte moving tensor.
- DVE per unit: mul prod=xh*wh (2x, ~2.2us) + two fp16 halving TTs +
  32-wide fp32 tensor_reduce (~3.1us). Measured alternatives that LOSE:
  GP tensor_tensor is 3.3x slower than DVE even solo and concurrent
  GP+DVE TTs collapse ~4x (shared SBUF ports); tensor_reduce is 1
  elem/cycle regardless of dtypes so a flat mul+reduce is slower than
  the tree; custom DVE ops (fused mul+scan) are rejected by this
  container's walrus ("ISA wrong length"); SWDGE cast/accum DMAs move
  only ~120-270 GB/s and steal from the same ~410 GB/s DMA envelope.
- PE consumes xh DIRECTLY (not prod): y_raw[h,f] = sum_t e[t,h]*xh[t,f],
  host divides by the normalizer only — prod becomes a short-lived
  tree-only tensor (bufs=2) and a deep xh pool (6) decouples the
  HAM-throttled PE (50%-duty windows) from the DVE stream, which is the
  critical engine (~88us busy vs the ~82us stream).
- exp on ScalarE -> e bf16 (logits reach +63: shift-invariant softmax
  needs no max subtraction; e^63 fits fp32/bf16). PE: lhsT = e bf16 x
  rhs = xh fp16 (mixed 2-byte matmul, verified on HW; 1024-col merged
  MMs are invalid ISA - PSUM banks cap MM out at 512 fp32), fp32 PSUM
  accumulation over all 32 chunks per batch, all low halves then all
  high halves (bank alternation causes HAM re-throttle).
- Normalizer: one ones^T @ e matmul per unit into a [1, nch*8] PSUM row;
  the host finishes s[h] = sum_n.
- Batch-0's PSUM drain is deferred into the middle of batch 1; its
  stores go on qAct behind the mid-stream x loads (descriptors are
  enqueued only after their data is ready, so no head-of-line risk).
"""

import math

import numpy as np

import concourse.bass as bass
import concourse.mybir as mybir
import concourse.tile as tile
from concourse.bass_utils import run_bass_kernel_spmd

B, T, F = 16, 4096, 1024
H, V, KD = 8, 128, 64
NCORES = 8
BL = B // NCORES            # batches per core: 2
NCH = 4                     # max 128-row chunks per unit
NCHUNKS = T // 128          # 32
FP32 = mybir.dt.float32
FP16 = mybir.dt.float16
BF16 = mybir.dt.bfloat16


# Work items per batch: (first-128-chunk, n-chunks, dma-queue engine).
# Small units head batch 0 (prime DVE early: its ~88us busy stream is the
# end-to-end critical path) and tail batch 1 (short final serial chain).
def _items_for(b):
    if b == 0:
        return [
            (0, 1, "sy"), (1, 1, "sc"), (2, 2, "sy"), (4, 4, "sc"),
            (8, 4, "sy"), (12, 4, "sc"), (16, 4, "sy"), (20, 4, "sy"),
            (24, 4, "sc"), (28, 4, "sy"),
        ]
    return [
        (0, 4, "sy"), (4, 4, "sc"), (8, 4, "sy"), (12, 4, "sy"),
        (16, 4, "sc"), (20, 4, "sy"), (24, 4, "sy"),
        (28, 2, "sc"), (30, 1, "sy"), (31, 1, "sy"),
    ]


def _build_nc():
    nc = bass.Bass()
    x_d = nc.declare_dram_parameter("x", [BL, T, F], FP32, isOutput=False)
    wh_d = nc.declare_dram_parameter("wh", [128, F], FP16, isOutput=False)
    y_d = nc.declare_dram_parameter("y", [BL, H, F], FP32, isOutput=True)
    # Raw per-(n,h) normalizer sums; the host finishes s[h] = sum_n and
    # divides (cheaper than an on-chip transpose + reciprocal on the tail).
    s_d = nc.declare_dram_parameter("s", [BL, 1, NCH * H], FP32, isOutput=True)

    with tile.TileContext(nc) as tc:
        with (
            tc.tile_pool(name="const", bufs=1) as const_pool,
            tc.tile_pool(name="xin", bufs=5) as xpool,
            tc.tile_pool(name="xh", bufs=6) as xhpool,
            tc.tile_pool(name="prod", bufs=2) as ppool,
            tc.tile_pool(name="half", bufs=2) as hpool,
            tc.tile_pool(name="quar", bufs=2) as qpool,
            tc.tile_pool(name="small", bufs=10) as small,
            tc.tile_pool(name="acc", bufs=1, space="PSUM") as psum_pool,
        ):
            # Weight row loads once on qAct ahead of any x load there.
            wh_sb = const_pool.tile([128, F], FP16)
            nc.scalar.dma_start(out=wh_sb, in_=wh_d[:, :])
            ones_sb = const_pool.tile([128, 1], BF16)
            nc.vector.memset(ones_sb, 1.0)
            # Touch Exp once at t=0 so the activation table loads during the
            # DMA ramp instead of on the first real exp's critical path.
            warm_sb = const_pool.tile([1, 1], FP32)
            nc.scalar.activation(
                out=warm_sb,
                in_=ones_sb[0:1, :].bitcast(BF16),
                func=mybir.ActivationFunctionType.Exp,
            )

            def emit_dma(b, ch0, nch, q):
                """x load for one unit, emitted two units ahead so both
                HWDGE rings stay fed through the ramp."""
                xt = xpool.tile([128, NCH, F], FP32, name="xt")
                xt_v = xt[:, :nch, :]
                eng = nc.sync if q == "sy" else nc.scalar
                eng.dma_start(
                    out=xt_v,
                    in_=x_d[
                        b, ch0 * 128 : (ch0 + nch) * 128, :
                    ].rearrange("(n p) f -> p n f", p=128),
                )
                return xt

            def emit_convert(nch, xt):
                """fp32->fp16 convert, emitted one unit ahead of the body so
                ScalarE's in-order stream never couples a convert behind the
                previous unit's exp (which waits on DVE)."""
                xh = xhpool.tile([128, NCH, F], FP16, name="xh")
                nc.scalar.activation(
                    out=xh[:, :nch, :],
                    in_=xt[:, :nch, :],
                    func=mybir.ActivationFunctionType.Copy,
                )
                return xh

            def emit_body(b, ch0, nch, xh, pooled_ps, s_ps, first, last):
                xh_v = xh[:, :nch, :]
                prod = ppool.tile([128, NCH, F], FP16, name="prod")
                prod_v = prod[:, :nch, :]
                wh_bc = bass.AP(
                    tensor=wh_sb.tensor,
                    offset=wh_sb.offset,
                    ap=[wh_sb.ap[0], [0, nch], wh_sb.ap[1]],
                )
                nc.vector.tensor_mul(prod_v, xh_v, wh_bc)
                # Grouped logit reduce over v=128, DVE-only: two fp16
                # halving TTs in 2x mode, then a 32-wide fp32 reduce. fp16
                # partial sums of x*w terms add <=1e-3 to logits.
                prod_hv = prod_v.rearrange("p n (h v) -> p n h v", v=V)
                half_t = hpool.tile([128, NCH, H, V // 2], FP16, name="half_t")
                quar_t = qpool.tile([128, NCH, H, V // 4], FP16, name="quar_t")
                with nc.allow_low_precision(
                    reason="fp16 pair sums of x*w; logits stay fp32 after"
                ):
                    nc.vector.tensor_add(
                        half_t[:, :nch, :, :],
                        prod_hv[:, :, :, 0 : V // 2],
                        prod_hv[:, :, :, V // 2 : V],
                    )
                    nc.vector.tensor_add(
                        quar_t[:, :nch, :, :],
                        half_t[:, :nch, :, 0 : V // 4],
                        half_t[:, :nch, :, V // 4 : V // 2],
                    )
                logits_u = small.tile([128, NCH, H], FP32, name="logits_u")
                nc.vector.tensor_reduce(
                    logits_u[:, :nch, :],
                    quar_t[:, :nch, :, :],
                    axis=mybir.AxisListType.X,
                    op=mybir.AluOpType.add,
                )
                e_u = small.tile([128, NCH, H], BF16, name="e_u")
                nc.scalar.activation(
                    out=e_u[:, :nch, :],
                    in_=logits_u[:, :nch, :],
                    func=mybir.ActivationFunctionType.Exp,
                )
                # Group matmuls by PSUM bank (all low halves, then all high
                # halves): per-MM bank alternation causes HAM re-throttle
                # and blocks MM pipelining. rhs is xh (not prod): the w
                # factor is folded out on the host via the logits instead,
                # so prod's lifetime ends at the tree and PE backpressure
                # never reaches the DVE through the prod pool.
                for half in range(2):
                    lo, hi = half * 512, half * 512 + 512
                    for n in range(nch):
                        ch = ch0 + n
                        nc.tensor.matmul(
                            pooled_ps[:, lo:hi],
                            e_u[:, n, :],
                            xh[:, n, lo:hi],
                            start=ch == 0,
                            stop=ch == NCHUNKS - 1,
                        )
                # One normalizer matmul per unit: ones^T @ e gives the
                # per-(n,h) partial sums as a [1, nch*8] PSUM row; the host
                # finishes the n-sum (units with nch<4 just fold their
                # chunks into the low n slots — still a complete sum).
                nc.tensor.matmul(
                    s_ps[:, 0 : nch * H],
                    ones_sb,
                    e_u[:, :nch, :],
                    start=first,
                    stop=last,
                )

            def emit_drain(b, pooled_ps, s_ps):
                y_sb = small.tile([H, F], FP32, name="y_sb")
                nc.scalar.activation(
                    out=y_sb,
                    in_=pooled_ps,
                    func=mybir.ActivationFunctionType.Copy,
                )
                s_sb = small.tile([1, NCH * H], FP32, name="s_sb")
                nc.vector.tensor_copy(s_sb, s_ps)
                # qAct: a store on the in-order qSP would stall the next x
                # loads behind the PSUM drain.
                nc.scalar.dma_start(out=y_d[b], in_=y_sb)
                nc.scalar.dma_start(out=s_d[b], in_=s_sb)

            # Flatten both batches into one software-pipelined stream:
            # dma(i+2) and convert(i+1) are emitted before body(i).
            sched = []
            for b in range(BL):
                pooled_ps = psum_pool.tile([H, F], FP32, name=f"pooled{b}")
                s_ps = psum_pool.tile([1, NCH * H], FP32, name=f"s{b}")
                items = _items_for(b)
                for it_idx, (ch0, nch, q) in enumerate(items):
                    sched.append(
                        (b, ch0, nch, q, pooled_ps, s_ps,
                         it_idx == 0, it_idx == len(items) - 1)
                    )

            n_u = len(sched)
            xts = [None] * n_u
            xhs = [None] * n_u
            for i in range(min(2, n_u)):
                bi, c0, nc_i, qi = sched[i][0], sched[i][1], sched[i][2], sched[i][3]
                xts[i] = emit_dma(bi, c0, nc_i, qi)
            xhs[0] = emit_convert(sched[0][2], xts[0])
            pending_drain = None
            for i in range(n_u):
                if i + 2 < n_u:
                    bi, c0, nc_i, qi = (
                        sched[i + 2][0], sched[i + 2][1],
                        sched[i + 2][2], sched[i + 2][3],
                    )
                    xts[i + 2] = emit_dma(bi, c0, nc_i, qi)
                if i + 1 < n_u:
                    xhs[i + 1] = emit_convert(sched[i + 1][2], xts[i + 1])
                b, ch0, nch, q, pooled_ps, s_ps, first, last = sched[i]
                emit_body(b, ch0, nch, xhs[i], pooled_ps, s_ps, first, last)
                if last:
                    if b == BL - 1:
                        emit_drain(b, pooled_ps, s_ps)
                    else:
                        # Defer this batch's PSUM drain: emitted mid-way
                        # through the next batch so Scalar's in-order stream
                        # doesn't couple the next batch's converts behind
                        # this batch's PSUM stop (which waits on the PE).
                        pending_drain = (b, pooled_ps, s_ps)
                elif pending_drain is not None and ch0 >= 12:
                    emit_drain(*pending_drain)
                    pending_drain = None
    return nc


def _split_multiwaits(nc, limit=1):
    """This container's walrus accepts at most `limit` sync-wait commands per
    instruction ("Too many sync wait commands" otherwise). Tile attaches up to
    ~12. Move excess waits onto preceding same-engine NoOps — semantics are
    unchanged (waits are AND conditions that block the engine either way)."""
    for fn in nc.m.functions:
        for blk in fn.blocks:
            new = []
            for inst in blk.instructions:
                si = getattr(inst, "sync_info", None)
                ow = list(si.on_wait) if si is not None and si.on_wait else []
                if len(ow) > limit:
                    extra, keep = ow[:-limit], ow[-limit:]
                    for i in range(0, len(extra), limit):
                        new.append(
                            mybir.InstNoOp(
                                name=f"{inst.name}-wsplit{i}",
                                engine=inst.engine,
                                ins=[],
                                outs=[],
                                sync_info=mybir.SyncInfo(
                                    on_wait=extra[i : i + limit], on_update=[]
                                ),
                            )
                        )
                    inst.sync_info = mybir.SyncInfo(
                        on_wait=keep, on_update=si.on_update
                    )
                new.append(inst)
            blk.instructions = new


_NC = None


def _get_nc():
    global _NC
    if _NC is None:
        _NC = _build_nc()
        _split_multiwaits(_NC)
    return _NC


def _fold_weights(Q, key_p):
    w = np.einsum(
        "hvk,hk->hv", np.asarray(Q, np.float32), np.asarray(key_p, np.float32)[:, :, 0]
    ) / np.float32(math.sqrt(KD))
    return w.reshape(H * V).astype(np.float32)


def _run(x, Q, key_p, trace=False, tmpdir=None):
    x = np.ascontiguousarray(np.asarray(x, np.float32))
    w_flat = _fold_weights(Q, key_p)
    wh = np.tile(w_flat.reshape(1, H * V), (128, 1)).astype(np.float16)
    nc = _get_nc()
    in_maps = [
        {"x": x[c * BL : (c + 1) * BL], "wh": wh}
        for c in range(NCORES)
    ]
    res = run_bass_kernel_spmd(
        nc, in_maps, list(range(NCORES)), trace=trace, tmpdir=tmpdir
    )
    # Kernel returns raw sum_t e*xh plus per-(n,h) normalizer partials; the
    # host finishes s and divides (w never touches the pooled values: it is
    # folded into the logits only).
    y = np.empty((B, F), np.float32)
    for c in range(NCORES):
        yc = res.results[c]["y"]  # [BL, H, F]
        sc = res.results[c]["s"]  # [BL, 1, NCH*H]
        for b in range(BL):
            s8 = sc[b, 0].reshape(NCH, H).sum(0, dtype=np.float32)
            for h in range(H):
                sl = slice(h * V, (h + 1) * V)
                y[c * BL + b, sl] = yc[b, h, sl] / s8[h]
    return y, res


def kernel(**inputs):
    y, _ = _run(inputs["x"], inputs["Q"], inputs["key_p"])
    return y
